# revision 10
# baseline (speedup 1.0000x reference)
"""Trainium2 Bass kernel for nn_Attention (b=8, n=1024, dim=768, heads=12).

Sharding: data-parallel over batch — 8 batch elements -> 8 NeuronCores.
Each core runs full attention for one [1024, 768] slice; weights replicated.

Per-core dataflow (all matmuls float32r, full PE rate at N>=256):
  x [n,c] --PE transpose--> xT [c,n]
  qT,kT = (w_qkv[:, :1536]).T @ x  in [d, n] head-pair layout
  V     = x @ w_qkv[:, 1536:]     in [n, d] layout, + ones column (denom fold)
  per head: ST[j,i] = kT.T @ qT ; E = exp(SCALE*ST) (ACT, PSUM->SBUF)
            O'[65, i] = [V|1].T @ E  (row 64 = softmax denominators)
            attnT[d, i] = O'[0:64] * (1/O'[64])  (gpsimd partition-broadcast)
  out[i, e] = attnT.T @ w_out + b_out
"""

import numpy as np
from contextlib import ExitStack

import concourse.bacc as bacc
import concourse.mybir as mybir
import concourse.tile as tile
from concourse.bass import ds, ts
from concourse.bass_utils import run_bass_kernel_spmd
from concourse.masks import make_identity

P = 128
N_CORES = 8
N_TOK = 1024
DIM = 768
H = 12
HD = 64
SCALE = 1.0 / (DIM ** 0.5)
F32 = mybir.dt.float32
F32R = mybir.dt.float32r
EXP = mybir.ActivationFunctionType.Exp

C_T = DIM // P          # 6  c-tiles
N_T = N_TOK // P        # 8  token tiles
I_HALF = 512            # i free-dim chunk for matmuls


def _emit_body(nc, tc, ctx, pools, dram, skip=()):
    x_d, wqkv_d, wout_d, bout_d, out_d = dram
    const, persist, xpool, wpool, expool, spool, outpool, psA, psO = pools

    wqkv_r = wqkv_d.rearrange("(o p) f -> p o f", p=P)
    wout_r = wout_d.rearrange("(o p) f -> p o f", p=P)

    # ---- constants ----
    identity = const.tile([P, P], F32, tag="ident")
    make_identity(nc, identity[:])
    b_row = const.tile([1, DIM], F32, tag="brow")
    nc.sync.dma_start(b_row[:], bout_d[None, :])
    bias_bc = const.tile([P, DIM], F32, tag="bias")
    nc.gpsimd.partition_broadcast(bias_bc[:], b_row[:])

    # ---- persistent tensors ----
    xT = persist.tile([P, C_T, N_TOK], F32R, tag="big6", name="xT")
    qkT = persist.tile([P, 6, 2, N_TOK], F32R, tag="qkT")   # [pair, q/k, n]
    vplus = persist.tile([P, N_T, H, HD + 1], F32R, tag="vplus")
    wv_sb = persist.tile([P, C_T, DIM], F32R, tag="wv")
    wo_sb = persist.tile([P, C_T, DIM], F32R, tag="wo")

    nc.sync.dma_start(wv_sb[:], wqkv_r[:, :, ds(2 * DIM, DIM)].bitcast(F32R))
    nc.sync.dma_start(wo_sb[:], wout_r[:].bitcast(F32R))
    nc.vector.memset(vplus[:, :, :, ds(HD, 1)].bitcast(F32), 1.0)

    # ---- phase 1: load x strips, PE-transpose into xT ----
    for it in range(N_T) if "p1" not in skip else ():
        xs = xpool.tile([P, DIM], F32, tag="xs")
        nc.sync.dma_start(xs[:], x_d[ts(it, P), :])
        for c in range(0, C_T, 2):
            pst = psA.tile([P, 2 * I_HALF], F32, tag="ps")
            nc.tensor.transpose(pst[:, 0:P], xs[:, ts(c, P)], identity[:])
            nc.tensor.transpose(pst[:, I_HALF:I_HALF + P], xs[:, ts(c + 1, P)], identity[:])
            nc.vector.tensor_copy(xT[:, c, ts(it, P)], pst[:, 0:P])
            nc.vector.tensor_copy(xT[:, c + 1, ts(it, P)], pst[:, I_HALF:I_HALF + P])

    # ---- phase 2a: q/k projection -> qkT ----
    for fi in range(12) if "p2a" not in skip else ():            # f-tiles over first 1536 cols of w_qkv
        pair, qk = fi % 6, fi // 6
        wt = wpool.tile([P, C_T, P], F32R, tag="wqk")
        nc.sync.dma_start(wt[:], wqkv_r[:, :, ds(fi * P, P)].bitcast(F32R))
        ps = psA.tile([P, 2 * I_HALF], F32, tag="ps")
        for k in range(C_T):
            for nh in range(2):
                nc.tensor.matmul(
                    ps[:, ds(nh * I_HALF, I_HALF)], wt[:, k],
                    xT[:, k, ds(nh * I_HALF, I_HALF)],
                    start=(k == 0), stop=(k == C_T - 1))
        nc.vector.tensor_copy(qkT[:, pair, qk, :], ps[:])

    # ---- phase 2b: V projection -> vplus[..., 0:64] ----
    for jt in range(N_T) if "p2b" not in skip else ():
        ps = psA.tile([P, 2 * I_HALF], F32, tag="ps")
        for k in range(C_T):
            for off, w in ((0, 512), (512, 256)):
                nc.tensor.matmul(
                    ps[:, ds(off, w)], xT[:, k, ts(jt, P)], wv_sb[:, k, ds(off, w)],
                    start=(k == 0), stop=(k == C_T - 1))
        nc.vector.tensor_copy(
            vplus[:, jt, :, ds(0, HD)],
            ps[:, :DIM].rearrange("p (h d) -> p h d", d=HD))

    # ---- phase 3: attention per head-pair ----
    attnT = persist.tile([P, C_T, N_TOK], F32R, tag="big6", name="attnT")
    for pr in range(6) if "p3" not in skip else ():
        o_ps = {}
        for sub in range(2):
            h = 2 * pr + sub
            o_ps[h] = psO.tile([P, 2 * I_HALF], F32, tag="po", name=f"po_{h}")
        for jt in range(N_T):
            sts = [psA.tile([P, 2 * I_HALF], F32, tag="ps", name=f"st_{pr}_{jt}_{sub}")
                   for sub in range(2)]
            # interleave row-groups so the pair co-runs on the PE array
            for i2 in range(2):
                for sub in range(2):
                    b0 = HD * sub
                    nc.tensor.matmul(
                        sts[sub][:, ds(i2 * I_HALF, I_HALF)],
                        qkT[b0:b0 + HD, pr, 1, ts(jt, P)],
                        qkT[b0:b0 + HD, pr, 0, ds(i2 * I_HALF, I_HALF)],
                        start=True, stop=True, tile_position=(b0, 0))
            stage = expool.tile([P, 2, N_TOK], F32, tag="stg", name=f"stg_{pr}_{jt}")
            for sub in range(2):
                nc.vector.tensor_copy(stage[:, sub, :], sts[sub][:])
            es = expool.tile([P, 2, N_TOK], F32R, tag="es", name=f"es_{pr}_{jt}")
            nc.scalar.activation(es[:], stage[:], EXP, scale=SCALE)
            for sub in range(2):
                h = 2 * pr + sub
                for i2 in range(2):
                    nc.tensor.matmul(
                        o_ps[h][:HD + 1, ds(i2 * I_HALF, I_HALF)], vplus[:, jt, h],
                        es[:, sub, ds(i2 * I_HALF, I_HALF)],
                        start=(jt == 0), stop=(jt == N_T - 1))
        # stage O' to SBUF (frees PSUM), then normalize off the critical path
        for sub in range(2):
            h = 2 * pr + sub
            ostg = spool.tile([HD + 1, N_TOK], F32, tag="ostg", name=f"ostg_{h}")
            nc.vector.tensor_copy(ostg[:], o_ps[h][:HD + 1, :])
            rec = spool.tile([1, N_TOK], F32, tag="rec")
            nc.vector.reciprocal(rec[:], ostg[HD:HD + 1, :])
            rb = spool.tile([HD, N_TOK], F32, tag="rb")
            nc.gpsimd.partition_broadcast(rb[:], rec[:])
            nc.vector.tensor_mul(
                attnT[HD * sub:HD * (sub + 1), h // 2, :],
                ostg[0:HD, :], rb[:])

    # ---- phase 4: output projection + bias ----
    for it in range(N_T) if "p4" not in skip else ():
        ps = psA.tile([P, 2 * I_HALF], F32, tag="ps")
        for k in range(C_T):
            for off, w in ((0, 512), (512, 256)):
                nc.tensor.matmul(
                    ps[:, ds(off, w)], attnT[:, k, ts(it, P)], wo_sb[:, k, ds(off, w)],
                    start=(k == 0), stop=(k == C_T - 1))
        os = outpool.tile([P, DIM], F32, tag="os")
        nc.vector.tensor_add(os[:], ps[:, :DIM], bias_bc[:])
        nc.sync.dma_start(out_d[ts(it, P), :], os[:])


def build_nc(reps: int = 1, timing_mode: bool = False, skip=()):
    nc = bacc.Bacc("TRN2", target_bir_lowering=False, debug=False)
    if timing_mode:
        # device-resident garbage inputs: measure kernel exec, not host I/O
        x_d = nc.dram_tensor("x", [N_TOK, DIM], F32).ap()
        wqkv_d = nc.dram_tensor("w_qkv", [DIM, 3 * DIM], F32).ap()
        wout_d = nc.dram_tensor("w_out", [DIM, DIM], F32).ap()
        bout_d = nc.dram_tensor("b_out", [DIM], F32).ap()
        out_d = nc.dram_tensor("out", [N_TOK, DIM], F32).ap()
        dummy_in = nc.dram_tensor("dummy_in", [1, 1], F32, kind="ExternalInput").ap()
        tiny_out = nc.dram_tensor("tiny_out", [1, 1], F32, kind="ExternalOutput").ap()
    else:
        x_d = nc.dram_tensor("x", [N_TOK, DIM], F32, kind="ExternalInput").ap()
        wqkv_d = nc.dram_tensor("w_qkv", [DIM, 3 * DIM], F32, kind="ExternalInput").ap()
        wout_d = nc.dram_tensor("w_out", [DIM, DIM], F32, kind="ExternalInput").ap()
        bout_d = nc.dram_tensor("b_out", [DIM], F32, kind="ExternalInput").ap()
        out_d = nc.dram_tensor("out", [N_TOK, DIM], F32, kind="ExternalOutput").ap()
    dram = (x_d, wqkv_d, wout_d, bout_d, out_d)

    with ExitStack() as ctx:
        tc = ctx.enter_context(tile.TileContext(nc))
        const = ctx.enter_context(tc.tile_pool(name="const", bufs=1))
        persist = ctx.enter_context(tc.tile_pool(name="persist", bufs=1))
        xpool = ctx.enter_context(tc.tile_pool(name="xpool", bufs=2))
        wpool = ctx.enter_context(tc.tile_pool(name="wpool", bufs=3))
        expool = ctx.enter_context(tc.tile_pool(name="expool", bufs=2))
        spool = ctx.enter_context(tc.tile_pool(name="spool", bufs=1))
        outpool = ctx.enter_context(tc.tile_pool(name="outpool", bufs=3))
        psA = ctx.enter_context(tc.tile_pool(name="psA", bufs=2, space="PSUM"))
        psO = ctx.enter_context(tc.tile_pool(name="psO", bufs=2, space="PSUM"))
        pools = (const, persist, xpool, wpool, expool, spool, outpool, psA, psO)

        if reps == 1:
            _emit_body(nc, tc, ctx, pools, dram, skip=skip)
        else:
            with tc.For_i(0, reps, 1):
                _emit_body(nc, tc, ctx, pools, dram, skip=skip)
        if timing_mode:
            tz = const.tile([1, 1], F32, tag="tz")
            nc.sync.dma_start(tz[:], dummy_in[:])
            nc.sync.dma_start(tiny_out[:], tz[:])

    nc.compile()
    return nc


_NC_CACHE = {}


def kernel(**inputs) -> np.ndarray:
    x = np.ascontiguousarray(np.asarray(inputs["x"], dtype=np.float32))
    w_qkv = np.ascontiguousarray(np.asarray(inputs["w_qkv"], dtype=np.float32))
    w_out = np.ascontiguousarray(np.asarray(inputs["w_out"], dtype=np.float32))
    b_out = np.ascontiguousarray(np.asarray(inputs["b_out"], dtype=np.float32))

    if "nc" not in _NC_CACHE:
        _NC_CACHE["nc"] = build_nc(reps=1)
    nc = _NC_CACHE["nc"]

    in_maps = [
        {"x": x[c], "w_qkv": w_qkv, "w_out": w_out, "b_out": b_out}
        for c in range(N_CORES)
    ]
    res = run_bass_kernel_spmd(nc, in_maps, core_ids=list(range(N_CORES)))
    out = np.stack([res.results[c]["out"] for c in range(N_CORES)], axis=0)
    return out.astype(np.float32)


# revision 11
# speedup vs baseline: 1.1436x; 1.1436x over previous
"""Trainium2 Bass kernel for nn_Attention (b=8, n=1024, dim=768, heads=12).

Sharding: data-parallel over batch — 8 batch elements -> 8 NeuronCores.
Each core runs full attention for one [1024, 768] slice; weights replicated.

Per-core dataflow (all matmuls float32r, full PE rate at N>=256):
  x [n,c] --PE transpose--> xT [c,n]
  qT,kT = (w_qkv[:, :1536]).T @ x  in [d, n] head-pair layout
  V     = x @ w_qkv[:, 1536:]     in [n, d] layout, + ones column (denom fold)
  per head: ST[j,i] = kT.T @ qT ; E = exp(SCALE*ST) (ACT, PSUM->SBUF)
            O'[65, i] = [V|1].T @ E  (row 64 = softmax denominators)
            attnT[d, i] = O'[0:64] * (1/O'[64])  (gpsimd partition-broadcast)
  out[i, e] = attnT.T @ w_out + b_out
"""

import numpy as np
from contextlib import ExitStack

import concourse.bacc as bacc
import concourse.mybir as mybir
import concourse.tile as tile
from concourse.bass import ds, ts
from concourse.bass_utils import run_bass_kernel_spmd
from concourse.masks import make_identity

P = 128
N_CORES = 8
N_TOK = 1024
DIM = 768
H = 12
HD = 64
SCALE = 1.0 / (DIM ** 0.5)
F32 = mybir.dt.float32
F32R = mybir.dt.float32r
EXP = mybir.ActivationFunctionType.Exp

C_T = DIM // P          # 6  c-tiles
N_T = N_TOK // P        # 8  token tiles
I_HALF = 512            # i free-dim chunk for matmuls


def _emit_body(nc, tc, ctx, pools, dram, skip=()):
    x_d, wqkv_d, wout_d, bout_d, out_d = dram
    const, persist, xpool, wpool, expool, spool, outpool, psA, psO = pools

    wqkv_r = wqkv_d.rearrange("(o p) f -> p o f", p=P)
    wout_r = wout_d.rearrange("(o p) f -> p o f", p=P)

    # ---- constants ----
    identity = const.tile([P, P], F32, tag="ident")
    make_identity(nc, identity[:])
    b_row = const.tile([1, DIM], F32, tag="brow")
    nc.sync.dma_start(b_row[:], bout_d[None, :])
    bias_bc = const.tile([P, DIM], F32, tag="bias")
    nc.gpsimd.partition_broadcast(bias_bc[:], b_row[:])

    # ---- persistent tensors ----
    xT = persist.tile([P, C_T, N_TOK], F32R, tag="big6", name="xT")
    qkT = persist.tile([P, 6, 2, N_TOK], F32R, tag="qkT")   # [pair, q/k, n]
    vplus = persist.tile([P, N_T, H, HD + 1], F32R, tag="vplus")
    wv_sb = persist.tile([P, C_T, DIM], F32R, tag="wv")
    wo_sb = persist.tile([P, C_T, DIM], F32R, tag="wo")

    nc.sync.dma_start(wv_sb[:], wqkv_r[:, :, ds(2 * DIM, DIM)].bitcast(F32R))
    nc.sync.dma_start(wo_sb[:], wout_r[:].bitcast(F32R))
    nc.vector.memset(vplus[:, :, :, ds(HD, 1)].bitcast(F32), 1.0)

    # ---- phase 1: load x strips, PE-transpose into xT ----
    for it in range(N_T) if "p1" not in skip else ():
        xs = xpool.tile([P, DIM], F32, tag="xs")
        nc.sync.dma_start(xs[:], x_d[ts(it, P), :])
        for c in range(0, C_T, 2):
            pst = psA.tile([P, 2 * I_HALF], F32, tag="ps")
            nc.tensor.transpose(pst[:, 0:P], xs[:, ts(c, P)], identity[:])
            nc.tensor.transpose(pst[:, I_HALF:I_HALF + P], xs[:, ts(c + 1, P)], identity[:])
            nc.vector.tensor_copy(xT[:, c, ts(it, P)], pst[:, 0:P])
            nc.vector.tensor_copy(xT[:, c + 1, ts(it, P)], pst[:, I_HALF:I_HALF + P])

    # ---- phase 2a: q/k projection -> qkT ----
    for fi in range(12) if "p2a" not in skip else ():            # f-tiles over first 1536 cols of w_qkv
        pair, qk = fi % 6, fi // 6
        wt = wpool.tile([P, C_T, P], F32R, tag="wqk")
        nc.sync.dma_start(wt[:], wqkv_r[:, :, ds(fi * P, P)].bitcast(F32R))
        ps = psA.tile([P, 2 * I_HALF], F32, tag="ps")
        for k in range(C_T):
            for nh in range(2):
                nc.tensor.matmul(
                    ps[:, ds(nh * I_HALF, I_HALF)], wt[:, k],
                    xT[:, k, ds(nh * I_HALF, I_HALF)],
                    start=(k == 0), stop=(k == C_T - 1))
        nc.vector.tensor_copy(qkT[:, pair, qk, :], ps[:])

    # ---- phase 2b: V projection -> vplus[..., 0:64] ----
    for jt in range(N_T) if "p2b" not in skip else ():
        ps = psA.tile([P, 2 * I_HALF], F32, tag="ps")
        for k in range(C_T):
            for off, w in ((0, 512), (512, 256)):
                nc.tensor.matmul(
                    ps[:, ds(off, w)], xT[:, k, ts(jt, P)], wv_sb[:, k, ds(off, w)],
                    start=(k == 0), stop=(k == C_T - 1))
        nc.vector.tensor_copy(
            vplus[:, jt, :, ds(0, HD)],
            ps[:, :DIM].rearrange("p (h d) -> p h d", d=HD))

    # ---- phase 3: attention per head-pair ----
    attnT = persist.tile([P, C_T, N_TOK], F32R, tag="big6", name="attnT")
    for pr in range(6) if "p3" not in skip else ():
        o_ps = {}
        for sub in range(2):
            h = 2 * pr + sub
            o_ps[h] = psO.tile([P, 2 * I_HALF], F32, tag="po", name=f"po_{h}")
        for jt in range(N_T):
            sts = [psA.tile([P, 2 * I_HALF], F32, tag="ps", name=f"st_{pr}_{jt}_{sub}")
                   for sub in range(2)]
            # interleave row-groups so the pair co-runs on the PE array
            for i2 in range(2):
                for sub in range(2):
                    b0 = HD * sub
                    nc.tensor.matmul(
                        sts[sub][:, ds(i2 * I_HALF, I_HALF)],
                        qkT[b0:b0 + HD, pr, 1, ts(jt, P)],
                        qkT[b0:b0 + HD, pr, 0, ds(i2 * I_HALF, I_HALF)],
                        start=True, stop=True, tile_position=(b0, 0))
            ess = []
            for sub in range(2):
                es = expool.tile([P, N_TOK], F32R, tag="es", name=f"es_{pr}_{jt}_{sub}")
                nc.scalar.activation(es[:], sts[sub][:], EXP, scale=SCALE)
                ess.append(es)
            for sub in range(2):
                h = 2 * pr + sub
                for i2 in range(2):
                    nc.tensor.matmul(
                        o_ps[h][:HD + 1, ds(i2 * I_HALF, I_HALF)], vplus[:, jt, h],
                        ess[sub][:, ds(i2 * I_HALF, I_HALF)],
                        start=(jt == 0), stop=(jt == N_T - 1))
        # stage O' to SBUF (frees PSUM), then normalize off the critical path
        for sub in range(2):
            h = 2 * pr + sub
            ostg = spool.tile([HD + 1, N_TOK], F32, tag="ostg", name=f"ostg_{h}")
            nc.vector.tensor_copy(ostg[:], o_ps[h][:HD + 1, :])
            rec = spool.tile([1, N_TOK], F32, tag="rec")
            nc.vector.reciprocal(rec[:], ostg[HD:HD + 1, :])
            rb = spool.tile([HD, N_TOK], F32, tag="rb")
            nc.gpsimd.partition_broadcast(rb[:], rec[:])
            nc.vector.tensor_mul(
                attnT[HD * sub:HD * (sub + 1), h // 2, :],
                ostg[0:HD, :], rb[:])

    # ---- phase 4: output projection + bias ----
    for it in range(N_T) if "p4" not in skip else ():
        ps = psA.tile([P, 2 * I_HALF], F32, tag="ps")
        for k in range(C_T):
            for off, w in ((0, 512), (512, 256)):
                nc.tensor.matmul(
                    ps[:, ds(off, w)], attnT[:, k, ts(it, P)], wo_sb[:, k, ds(off, w)],
                    start=(k == 0), stop=(k == C_T - 1))
        os = outpool.tile([P, DIM], F32, tag="os")
        nc.vector.tensor_add(os[:], ps[:, :DIM], bias_bc[:])
        nc.sync.dma_start(out_d[ts(it, P), :], os[:])


def build_nc(reps: int = 1, timing_mode: bool = False, skip=()):
    nc = bacc.Bacc("TRN2", target_bir_lowering=False, debug=False)
    if timing_mode:
        # device-resident garbage inputs: measure kernel exec, not host I/O
        x_d = nc.dram_tensor("x", [N_TOK, DIM], F32).ap()
        wqkv_d = nc.dram_tensor("w_qkv", [DIM, 3 * DIM], F32).ap()
        wout_d = nc.dram_tensor("w_out", [DIM, DIM], F32).ap()
        bout_d = nc.dram_tensor("b_out", [DIM], F32).ap()
        out_d = nc.dram_tensor("out", [N_TOK, DIM], F32).ap()
        dummy_in = nc.dram_tensor("dummy_in", [1, 1], F32, kind="ExternalInput").ap()
        tiny_out = nc.dram_tensor("tiny_out", [1, 1], F32, kind="ExternalOutput").ap()
    else:
        x_d = nc.dram_tensor("x", [N_TOK, DIM], F32, kind="ExternalInput").ap()
        wqkv_d = nc.dram_tensor("w_qkv", [DIM, 3 * DIM], F32, kind="ExternalInput").ap()
        wout_d = nc.dram_tensor("w_out", [DIM, DIM], F32, kind="ExternalInput").ap()
        bout_d = nc.dram_tensor("b_out", [DIM], F32, kind="ExternalInput").ap()
        out_d = nc.dram_tensor("out", [N_TOK, DIM], F32, kind="ExternalOutput").ap()
    dram = (x_d, wqkv_d, wout_d, bout_d, out_d)

    with ExitStack() as ctx:
        tc = ctx.enter_context(tile.TileContext(nc))
        const = ctx.enter_context(tc.tile_pool(name="const", bufs=1))
        persist = ctx.enter_context(tc.tile_pool(name="persist", bufs=1))
        xpool = ctx.enter_context(tc.tile_pool(name="xpool", bufs=2))
        wpool = ctx.enter_context(tc.tile_pool(name="wpool", bufs=3))
        expool = ctx.enter_context(tc.tile_pool(name="expool", bufs=4))
        spool = ctx.enter_context(tc.tile_pool(name="spool", bufs=1))
        outpool = ctx.enter_context(tc.tile_pool(name="outpool", bufs=3))
        psA = ctx.enter_context(tc.tile_pool(name="psA", bufs=2, space="PSUM"))
        psO = ctx.enter_context(tc.tile_pool(name="psO", bufs=2, space="PSUM"))
        pools = (const, persist, xpool, wpool, expool, spool, outpool, psA, psO)

        if reps == 1:
            _emit_body(nc, tc, ctx, pools, dram, skip=skip)
        else:
            with tc.For_i(0, reps, 1):
                _emit_body(nc, tc, ctx, pools, dram, skip=skip)
        if timing_mode:
            tz = const.tile([1, 1], F32, tag="tz")
            nc.sync.dma_start(tz[:], dummy_in[:])
            nc.sync.dma_start(tiny_out[:], tz[:])

    nc.compile()
    return nc


_NC_CACHE = {}


def kernel(**inputs) -> np.ndarray:
    x = np.ascontiguousarray(np.asarray(inputs["x"], dtype=np.float32))
    w_qkv = np.ascontiguousarray(np.asarray(inputs["w_qkv"], dtype=np.float32))
    w_out = np.ascontiguousarray(np.asarray(inputs["w_out"], dtype=np.float32))
    b_out = np.ascontiguousarray(np.asarray(inputs["b_out"], dtype=np.float32))

    if "nc" not in _NC_CACHE:
        _NC_CACHE["nc"] = build_nc(reps=1)
    nc = _NC_CACHE["nc"]

    in_maps = [
        {"x": x[c], "w_qkv": w_qkv, "w_out": w_out, "b_out": b_out}
        for c in range(N_CORES)
    ]
    res = run_bass_kernel_spmd(nc, in_maps, core_ids=list(range(N_CORES)))
    out = np.stack([res.results[c]["out"] for c in range(N_CORES)], axis=0)
    return out.astype(np.float32)


# revision 12
# speedup vs baseline: 1.4433x; 1.2621x over previous
"""Trainium2 Bass kernel for nn_Attention (b=8, n=1024, dim=768, heads=12).

Sharding: data-parallel over batch — 8 batch elements -> 8 NeuronCores.
Each core runs full attention for one [1024, 768] slice; weights replicated.

Per-core dataflow (all matmuls float32r, full PE rate at N>=256):
  x [n,c] --PE transpose--> xT [c,n]
  qT,kT = (w_qkv[:, :1536]).T @ x  in [d, n] head-pair layout
  V     = x @ w_qkv[:, 1536:]     in [n, d] layout, + ones column (denom fold)
  per head: ST[j,i] = kT.T @ qT ; E = exp(SCALE*ST) (ACT, PSUM->SBUF)
            O'[65, i] = [V|1].T @ E  (row 64 = softmax denominators)
            attnT[d, i] = O'[0:64] * (1/O'[64])  (gpsimd partition-broadcast)
  out[i, e] = attnT.T @ w_out + b_out
"""

import numpy as np
from contextlib import ExitStack

import concourse.bacc as bacc
import concourse.mybir as mybir
import concourse.tile as tile
from concourse.bass import ds, ts
from concourse.bass_utils import run_bass_kernel_spmd
from concourse.masks import make_identity

P = 128
N_CORES = 8
N_TOK = 1024
DIM = 768
H = 12
HD = 64
SCALE = 1.0 / (DIM ** 0.5)
F32 = mybir.dt.float32
F32R = mybir.dt.float32r
EXP = mybir.ActivationFunctionType.Exp

C_T = DIM // P          # 6  c-tiles
N_T = N_TOK // P        # 8  token tiles
I_HALF = 512            # i free-dim chunk for matmuls


def _emit_body(nc, tc, ctx, pools, dram, skip=()):
    x_d, wqkv_d, wout_d, bout_d, out_d = dram
    const, persist, xpool, wpool, expool, spool, outpool, psA, psO = pools

    wqkv_r = wqkv_d.rearrange("(o p) f -> p o f", p=P)
    wout_r = wout_d.rearrange("(o p) f -> p o f", p=P)

    # ---- constants ----
    identity = const.tile([P, P], F32, tag="ident")
    make_identity(nc, identity[:])
    b_row = const.tile([1, DIM], F32, tag="brow")
    nc.sync.dma_start(b_row[:], bout_d[None, :])
    bias_bc = const.tile([P, DIM], F32, tag="bias")
    nc.gpsimd.partition_broadcast(bias_bc[:], b_row[:])

    # ---- persistent tensors ----
    xT = persist.tile([P, C_T, N_TOK], F32R, tag="big6", name="xT")
    qkT = persist.tile([P, 6, 2, N_TOK], F32R, tag="qkT")   # [pair, q/k, n]
    vplus = persist.tile([P, N_T, H, HD + 1], F32R, tag="vplus")
    wv_sb = persist.tile([P, C_T, DIM], F32R, tag="wv")
    wo_sb = persist.tile([P, C_T, DIM], F32R, tag="wo")

    nc.sync.dma_start(wv_sb[:], wqkv_r[:, :, ds(2 * DIM, DIM)].bitcast(F32R))
    nc.sync.dma_start(wo_sb[:], wout_r[:].bitcast(F32R))
    nc.vector.memset(vplus[:, :, :, ds(HD, 1)].bitcast(F32), 1.0)

    # ---- phase 1: load x strips, PE-transpose into xT ----
    for it in range(N_T) if "p1" not in skip else ():
        xs = xpool.tile([P, DIM], F32, tag="xs")
        nc.sync.dma_start(xs[:], x_d[ts(it, P), :])
        for c in range(0, C_T, 2):
            pst = psA.tile([P, 2 * I_HALF], F32, tag="ps")
            nc.tensor.transpose(pst[:, 0:P], xs[:, ts(c, P)], identity[:])
            nc.tensor.transpose(pst[:, I_HALF:I_HALF + P], xs[:, ts(c + 1, P)], identity[:])
            nc.vector.tensor_copy(xT[:, c, ts(it, P)], pst[:, 0:P])
            nc.vector.tensor_copy(xT[:, c + 1, ts(it, P)], pst[:, I_HALF:I_HALF + P])

    # ---- phase 2a: q/k projection -> qkT ----
    for fi in range(12) if "p2a" not in skip else ():            # f-tiles over first 1536 cols of w_qkv
        pair, qk = fi % 6, fi // 6
        wt = wpool.tile([P, C_T, P], F32R, tag="wqk")
        nc.sync.dma_start(wt[:], wqkv_r[:, :, ds(fi * P, P)].bitcast(F32R))
        ps = psA.tile([P, 2 * I_HALF], F32, tag="ps")
        for k in range(C_T):
            for nh in range(2):
                nc.tensor.matmul(
                    ps[:, ds(nh * I_HALF, I_HALF)], wt[:, k],
                    xT[:, k, ds(nh * I_HALF, I_HALF)],
                    start=(k == 0), stop=(k == C_T - 1))
        nc.vector.tensor_copy(qkT[:, pair, qk, :], ps[:])

    # ---- phase 2b: V projection -> vplus[..., 0:64] ----
    for jt in range(N_T) if "p2b" not in skip else ():
        ps = psA.tile([P, 2 * I_HALF], F32, tag="ps")
        for k in range(C_T):
            for off, w in ((0, 512), (512, 256)):
                nc.tensor.matmul(
                    ps[:, ds(off, w)], xT[:, k, ts(jt, P)], wv_sb[:, k, ds(off, w)],
                    start=(k == 0), stop=(k == C_T - 1))
        nc.vector.tensor_copy(
            vplus[:, jt, :, ds(0, HD)],
            ps[:, :DIM].rearrange("p (h d) -> p h d", d=HD))

    # ---- phase 3: attention per head-pair ----
    attnT = persist.tile([P, C_T, N_TOK], F32R, tag="big6", name="attnT")
    for pr in range(6) if "p3" not in skip else ():
        o_ps = {}
        for sub in range(2):
            h = 2 * pr + sub
            o_ps[h] = psO.tile([P, 2 * I_HALF], F32, tag="po", name=f"po_{h}")
        for jt in range(N_T):
            sts = [psA.tile([P, 2 * I_HALF], F32, tag="ps", name=f"st_{pr}_{jt}_{sub}")
                   for sub in range(2)]
            # interleave row-groups so the pair co-runs on the PE array
            for i2 in range(2):
                for sub in range(2):
                    b0 = HD * sub
                    nc.tensor.matmul(
                        sts[sub][:, ds(i2 * I_HALF, I_HALF)],
                        qkT[b0:b0 + HD, pr, 1, ts(jt, P)],
                        qkT[b0:b0 + HD, pr, 0, ds(i2 * I_HALF, I_HALF)],
                        start=True, stop=True, tile_position=(b0, 0))
            ess = []
            for sub in range(2):
                es = expool.tile([P, N_TOK], F32R, tag="es", name=f"es_{pr}_{jt}_{sub}")
                nc.scalar.activation(es[:], sts[sub][:], EXP, scale=SCALE)
                ess.append(es)
            for sub in range(2):
                h = 2 * pr + sub
                for i2 in range(2):
                    nc.tensor.matmul(
                        o_ps[h][:HD + 1, ds(i2 * I_HALF, I_HALF)], vplus[:, jt, h],
                        ess[sub][:, ds(i2 * I_HALF, I_HALF)],
                        start=(jt == 0), stop=(jt == N_T - 1))
        # stage O' to SBUF (frees PSUM), then normalize off the critical path
        for sub in range(2):
            h = 2 * pr + sub
            ostg = spool.tile([HD + 1, N_TOK], F32, tag="ostg", name=f"ostg_{h}")
            nc.vector.tensor_copy(ostg[:], o_ps[h][:HD + 1, :])
            rec = spool.tile([1, N_TOK], F32, tag="rec")
            nc.vector.reciprocal(rec[:], ostg[HD:HD + 1, :])
            rb = spool.tile([HD, N_TOK], F32, tag="rb")
            nc.gpsimd.partition_broadcast(rb[:], rec[:])
            nc.vector.tensor_mul(
                attnT[HD * sub:HD * (sub + 1), h // 2, :],
                ostg[0:HD, :], rb[:])

    # ---- phase 4: output projection + bias ----
    for it in range(N_T) if "p4" not in skip else ():
        ps = psA.tile([P, 2 * I_HALF], F32, tag="ps")
        for k in range(C_T):
            for off, w in ((0, 512), (512, 256)):
                nc.tensor.matmul(
                    ps[:, ds(off, w)], attnT[:, k, ts(it, P)], wo_sb[:, k, ds(off, w)],
                    start=(k == 0), stop=(k == C_T - 1))
        os = outpool.tile([P, DIM], F32, tag="os")
        nc.vector.tensor_add(os[:], ps[:, :DIM], bias_bc[:])
        nc.sync.dma_start(out_d[ts(it, P), :], os[:])


def build_nc(reps: int = 1, timing_mode: bool = False, skip=()):
    nc = bacc.Bacc("TRN2", target_bir_lowering=False, debug=False)
    if timing_mode:
        # device-resident garbage inputs: measure kernel exec, not host I/O
        x_d = nc.dram_tensor("x", [N_TOK, DIM], F32).ap()
        wqkv_d = nc.dram_tensor("w_qkv", [DIM, 3 * DIM], F32).ap()
        wout_d = nc.dram_tensor("w_out", [DIM, DIM], F32).ap()
        bout_d = nc.dram_tensor("b_out", [DIM], F32).ap()
        out_d = nc.dram_tensor("out", [N_TOK, DIM], F32).ap()
        dummy_in = nc.dram_tensor("dummy_in", [1, 1], F32, kind="ExternalInput").ap()
        tiny_out = nc.dram_tensor("tiny_out", [1, 1], F32, kind="ExternalOutput").ap()
    else:
        x_d = nc.dram_tensor("x", [N_TOK, DIM], F32, kind="ExternalInput").ap()
        wqkv_d = nc.dram_tensor("w_qkv", [DIM, 3 * DIM], F32, kind="ExternalInput").ap()
        wout_d = nc.dram_tensor("w_out", [DIM, DIM], F32, kind="ExternalInput").ap()
        bout_d = nc.dram_tensor("b_out", [DIM], F32, kind="ExternalInput").ap()
        out_d = nc.dram_tensor("out", [N_TOK, DIM], F32, kind="ExternalOutput").ap()
    dram = (x_d, wqkv_d, wout_d, bout_d, out_d)

    with ExitStack() as ctx:
        tc = ctx.enter_context(tile.TileContext(nc))
        const = ctx.enter_context(tc.tile_pool(name="const", bufs=1))
        persist = ctx.enter_context(tc.tile_pool(name="persist", bufs=1))
        xpool = ctx.enter_context(tc.tile_pool(name="xpool", bufs=2))
        wpool = ctx.enter_context(tc.tile_pool(name="wpool", bufs=3))
        expool = ctx.enter_context(tc.tile_pool(name="expool", bufs=3))
        spool = ctx.enter_context(tc.tile_pool(name="spool", bufs=2))
        outpool = ctx.enter_context(tc.tile_pool(name="outpool", bufs=3))
        psA = ctx.enter_context(tc.tile_pool(name="psA", bufs=2, space="PSUM"))
        psO = ctx.enter_context(tc.tile_pool(name="psO", bufs=2, space="PSUM"))
        pools = (const, persist, xpool, wpool, expool, spool, outpool, psA, psO)

        if reps == 1:
            _emit_body(nc, tc, ctx, pools, dram, skip=skip)
        else:
            with tc.For_i(0, reps, 1):
                _emit_body(nc, tc, ctx, pools, dram, skip=skip)
        if timing_mode:
            tz = const.tile([1, 1], F32, tag="tz")
            nc.sync.dma_start(tz[:], dummy_in[:])
            nc.sync.dma_start(tiny_out[:], tz[:])

    nc.compile()
    return nc


_NC_CACHE = {}


def kernel(**inputs) -> np.ndarray:
    x = np.ascontiguousarray(np.asarray(inputs["x"], dtype=np.float32))
    w_qkv = np.ascontiguousarray(np.asarray(inputs["w_qkv"], dtype=np.float32))
    w_out = np.ascontiguousarray(np.asarray(inputs["w_out"], dtype=np.float32))
    b_out = np.ascontiguousarray(np.asarray(inputs["b_out"], dtype=np.float32))

    if "nc" not in _NC_CACHE:
        _NC_CACHE["nc"] = build_nc(reps=1)
    nc = _NC_CACHE["nc"]

    in_maps = [
        {"x": x[c], "w_qkv": w_qkv, "w_out": w_out, "b_out": b_out}
        for c in range(N_CORES)
    ]
    res = run_bass_kernel_spmd(nc, in_maps, core_ids=list(range(N_CORES)))
    out = np.stack([res.results[c]["out"] for c in range(N_CORES)], axis=0)
    return out.astype(np.float32)
